# revision 10
# baseline (speedup 1.0000x reference)
"""Causal self-attention on 8 TRN2 NeuronCores.

Sharding: B=4 batches x 16 heads -> 64 (b,h) pairs; core c handles batch
b=c//2 and head-group hg=c%2 (8 heads = 512 of the 1024 features).
Q/K/V projection weights are row-sliced per head group (column-sharded in
the x @ W.T sense), so each core computes its own (b, 8-head) slice of the
S x S attention without any cross-core communication.

Kernel layout choices:
- Matmuls contract over SBUF partitions, so X^T (and W^T) are materialized
  on-chip via PE transpose-mode matmuls (fp32 has no DMA transpose).
- All big matmuls run as float32r (full PE rate at free-dim 512, ~tf32
  precision).
- Scores are computed transposed, S^T[k, q] = (K^T)^T Q^T per 128-key chunk,
  so that softmax(S)@V becomes out^T[d, q] = V^T P^T with a 512-wide moving
  operand. Heads are processed in pairs: head parity picks partitions 0-63
  vs 64-127 (independent PE row groups -> the two K=64 matmuls overlap).
- No row-max subtraction: scaled scores are ~N(0,1), exp is safe in fp32.
  exp runs on ScalarE straight from PSUM with the attention-mask bias and
  1/sqrt(64) scale fused in; causal masking is a 0/1 multiply after exp.
- A ones-column appended to V makes the AV matmul also produce the softmax
  denominator (row 64 of the [65, 512] PSUM accumulator).
- Output heads are PE-transposed back to [seq, d] and normalized by the
  reciprocal of the denominator column on the way out.
"""

import sys

if "/opt/trn_rl_repo" not in sys.path:
    sys.path.insert(0, "/opt/trn_rl_repo")

import numpy as np

B, S, H, NH = 4, 2048, 1024, 16
HD = 64
NCORES = 8
F = 512  # features per core (8 heads)
NHEADS = 8  # heads per core
NPAIR = 4  # head pairs per core
HCH = H // 128  # 8 hidden chunks
SCH = S // 128  # 16 sequence chunks
P = 128

_CACHE = {}


def _build_bass():
    import concourse.tile as tile
    from concourse import bacc, mybir
    from contextlib import ExitStack

    f32 = mybir.dt.float32
    f32r = mybir.dt.float32r
    EXP = mybir.ActivationFunctionType.Exp
    ADD = mybir.AluOpType.add

    nc = bacc.Bacc("TRN2", target_bir_lowering=False, debug=False, num_devices=NCORES)

    x_d = nc.dram_tensor("x", [S, H], f32, kind="ExternalInput").ap()
    wq_d = nc.dram_tensor("wq", [F, H], f32, kind="ExternalInput").ap()
    wk_d = nc.dram_tensor("wk", [F, H], f32, kind="ExternalInput").ap()
    wv_d = nc.dram_tensor("wv", [F, H], f32, kind="ExternalInput").ap()
    bqt_d = nc.dram_tensor("bqt", [P, NPAIR], f32, kind="ExternalInput").ap()
    bkt_d = nc.dram_tensor("bkt", [P, NPAIR], f32, kind="ExternalInput").ap()
    bvb_d = nc.dram_tensor("bvb", [P, F], f32, kind="ExternalInput").ap()
    maskb_d = nc.dram_tensor("maskb", [P, SCH], f32, kind="ExternalInput").ap()
    cm_d = nc.dram_tensor("cm", [P, 4, 512], f32, kind="ExternalInput").ap()
    id_d = nc.dram_tensor("ident", [P, P], f32, kind="ExternalInput").ap()
    out_d = nc.dram_tensor("out", [S, F], f32, kind="ExternalOutput").ap()

    with tile.TileContext(nc) as tc, ExitStack() as ctx:
        const = ctx.enter_context(tc.tile_pool(name="const", bufs=1))
        ident = const.tile([P, P], f32, tag="ident")
        nc.sync.dma_start(ident[:], id_d[:])
        cm = const.tile([P, 4, 512], f32, tag="cm")
        nc.sync.dma_start(cm[:], cm_d[:])
        maskb = const.tile([P, SCH], f32, tag="maskb")
        nc.sync.dma_start(maskb[:], maskb_d[:])
        bqt = const.tile([P, NPAIR], f32, tag="bqt")
        nc.sync.dma_start(bqt[:], bqt_d[:])
        bkt = const.tile([P, NPAIR], f32, tag="bkt")
        nc.sync.dma_start(bkt[:], bkt_d[:])
        bvb = const.tile([P, F], f32, tag="bvb")
        nc.sync.dma_start(bvb[:], bvb_d[:])

        xt_pool = ctx.enter_context(tc.tile_pool(name="xt", bufs=1))
        xt = xt_pool.tile([P, HCH, S], f32r, tag="xt")  # X^T, 64KB/partition
        v_pool = ctx.enter_context(tc.tile_pool(name="v", bufs=1))
        v = v_pool.tile([P, SCH, NHEADS, HD + 1], f32r, tag="v")  # V + ones col

        stage = ctx.enter_context(tc.tile_pool(name="stage", bufs=2))
        trps = ctx.enter_context(tc.tile_pool(name="trps", bufs=2, space="PSUM"))
        mmps = ctx.enter_context(tc.tile_pool(name="mmps", bufs=2, space="PSUM"))
        ops_ = ctx.enter_context(tc.tile_pool(name="ops", bufs=2, space="PSUM"))
        wt_pool = ctx.enter_context(tc.tile_pool(name="wt", bufs=2))
        qkt_pool = ctx.enter_context(tc.tile_pool(name="qkt", bufs=2))
        p_pool = ctx.enter_context(tc.tile_pool(name="pp", bufs=3))
        ot_pool = ctx.enter_context(tc.tile_pool(name="ot", bufs=2))
        obuf = ctx.enter_context(tc.tile_pool(name="obuf", bufs=4))
        rec_pool = ctx.enter_context(tc.tile_pool(name="rec", bufs=4))

        def transpose_128(dst_ap, src_ap):
            # dst[128, 128] (SBUF) = src[128, 128].T via PE + DVE copyback
            tp = trps.tile([P, P], f32, tag="tr")
            nc.tensor.transpose(tp[:], src_ap, ident[:])
            nc.vector.tensor_copy(dst_ap, tp[:])

        # ---- X^T ----
        for si in range(SCH):
            xs = stage.tile([P, H], f32, tag="stage")
            nc.sync.dma_start(xs[:], x_d[si * 128 : (si + 1) * 128, :])
            for j in range(HCH):
                transpose_128(
                    xt[:, j, si * 128 : (si + 1) * 128],
                    xs[:, j * 128 : (j + 1) * 128],
                )

        # ---- V = X @ Wv_s^T + bv (plus ones column) ----
        with tc.tile_pool(name="wtv", bufs=1) as wtv_pool:
            wtv = wtv_pool.tile([P, HCH, F], f32r, tag="wtv")
            for rc in range(4):
                ws = stage.tile([P, H], f32, tag="stage")
                nc.sync.dma_start(ws[:], wv_d[rc * 128 : (rc + 1) * 128, :])
                for j in range(HCH):
                    transpose_128(
                        wtv[:, j, rc * 128 : (rc + 1) * 128],
                        ws[:, j * 128 : (j + 1) * 128],
                    )
            nc.vector.tensor_scalar(
                v[:, :, :, HD : HD + 1],
                bvb[:, 0:128].rearrange("p (a b c) -> p a b c", a=SCH, b=NHEADS),
                0.0,
                1.0,
                mybir.AluOpType.mult,
                mybir.AluOpType.add,
            )
            for si in range(SCH):
                ps = mmps.tile([P, 1024], f32, tag="mm")
                for j in range(HCH):
                    nc.tensor.matmul(
                        ps[:, 0:F],
                        xt[:, j, si * 128 : (si + 1) * 128],
                        wtv[:, j, :],
                        start=(j == 0),
                        stop=(j == HCH - 1),
                    )
                nc.vector.tensor_tensor(
                    v[:, si, :, 0:HD],
                    ps[:, 0:F].rearrange("p (h d) -> p h d", h=NHEADS),
                    bvb[:].rearrange("p (h d) -> p h d", h=NHEADS),
                    ADD,
                )

        # ---- per head-pair: project Q^T/K^T then attention ----
        for pr in range(NPAIR):
            h0, h1 = 2 * pr, 2 * pr + 1
            wtq = wt_pool.tile([P, HCH, P], f32r, tag="wtq")
            wtk = wt_pool.tile([P, HCH, P], f32r, tag="wtk")
            for wd, wt in ((wq_d, wtq), (wk_d, wtk)):
                ws = stage.tile([P, H], f32, tag="stage")
                nc.sync.dma_start(ws[:], wd[pr * 128 : (pr + 1) * 128, :])
                for j in range(HCH):
                    transpose_128(wt[:, j, :], ws[:, j * 128 : (j + 1) * 128])

            qt = qkt_pool.tile([P, S], f32r, tag="qt")
            kt = qkt_pool.tile([P, S], f32r, tag="kt")
            for wt, dst, bias in ((wtq, qt, bqt), (wtk, kt, bkt)):
                for st in range(4):
                    ps = mmps.tile([P, 1024], f32, tag="mm")
                    for j in range(HCH):
                        nc.tensor.matmul(
                            ps[:, 0:F],
                            wt[:, j, :],
                            xt[:, j, st * 512 : (st + 1) * 512],
                            start=(j == 0),
                            stop=(j == HCH - 1),
                        )
                    nc.vector.tensor_scalar_add(
                        dst[:, st * 512 : (st + 1) * 512],
                        ps[:, 0:F],
                        bias[:, pr : pr + 1],
                    )

            for qi in range(4):
                q0 = qi * 512
                nk = 4 * (qi + 1)
                oa = ops_.tile([P, F], f32, tag="o")
                ob = ops_.tile([P, F], f32, tag="o")
                for kc in range(nk):
                    ps = mmps.tile([P, 1024], f32, tag="mm")
                    nc.tensor.matmul(
                        ps[:, 0:512],
                        kt[0:64, kc * 128 : (kc + 1) * 128],
                        qt[0:64, q0 : q0 + 512],
                        start=True,
                        stop=True,
                    )
                    nc.tensor.matmul(
                        ps[:, 512:1024],
                        kt[64:128, kc * 128 : (kc + 1) * 128],
                        qt[64:128, q0 : q0 + 512],
                        start=True,
                        stop=True,
                    )
                    pt = p_pool.tile([P, 1024], f32r, tag="pt")
                    nc.scalar.activation(
                        pt[:], ps[:], EXP, bias=maskb[:, kc : kc + 1], scale=0.125
                    )
                    off = kc - 4 * qi
                    if off >= 0:
                        nc.vector.tensor_mul(
                            pt[:].rearrange("p (t q) -> p t q", t=2),
                            pt[:].rearrange("p (t q) -> p t q", t=2),
                            cm[:, off : off + 1, :].to_broadcast((P, 2, 512)),
                        )
                    nc.tensor.matmul(
                        oa[0 : HD + 1, :],
                        v[:, kc, h0, :],
                        pt[:, 0:512],
                        start=(kc == 0),
                        stop=(kc == nk - 1),
                    )
                    nc.tensor.matmul(
                        ob[0 : HD + 1, :],
                        v[:, kc, h1, :],
                        pt[:, 512:1024],
                        start=(kc == 0),
                        stop=(kc == nk - 1),
                    )
                for o_ps, h in ((oa, h0), (ob, h1)):
                    ot = ot_pool.tile([HD + 1, F], f32, tag="ot")
                    nc.vector.tensor_copy(ot[:], o_ps[0 : HD + 1, :])
                    for t in range(4):
                        tp = trps.tile([P, P], f32, tag="tr")
                        nc.tensor.transpose(
                            tp[:, 0 : HD + 1],
                            ot[:, t * 128 : (t + 1) * 128],
                            ident[0 : HD + 1, 0 : HD + 1],
                        )
                        rec = rec_pool.tile([P, 1], f32, tag="rec")
                        nc.vector.reciprocal(rec[:], tp[:, HD : HD + 1])
                        otile = obuf.tile([P, HD], f32, tag="ob")
                        nc.vector.tensor_scalar_mul(otile[:], tp[:, 0:HD], rec[:])
                        nc.sync.dma_start(
                            out_d[q0 + t * 128 : q0 + (t + 1) * 128, h * HD : (h + 1) * HD],
                            otile[:],
                        )

    nc.compile()
    return nc


def _get_nc():
    if "nc" not in _CACHE:
        _CACHE["nc"] = _build_bass()
    return _CACHE["nc"]


def _host_consts():
    if "consts" not in _CACHE:
        m = np.zeros((P, 4, 512), dtype=np.float32)
        qq = np.arange(512)[None, :]
        kk = np.arange(P)[:, None]
        for off in range(4):
            m[:, off, :] = (qq >= (kk + off * 128)).astype(np.float32)
        _CACHE["consts"] = {
            "cm": m,
            "ident": np.eye(P, dtype=np.float32),
        }
    return _CACHE["consts"]


def make_in_maps(inputs):
    hs = np.asarray(inputs["hidden_states"], dtype=np.float32)
    am = np.asarray(inputs["attention_mask"], dtype=np.float32)
    Wq = np.asarray(inputs["Wq"], dtype=np.float32)
    bq = np.asarray(inputs["bq"], dtype=np.float32)
    Wk = np.asarray(inputs["Wk"], dtype=np.float32)
    bk = np.asarray(inputs["bk"], dtype=np.float32)
    Wv = np.asarray(inputs["Wv"], dtype=np.float32)
    bv = np.asarray(inputs["bv"], dtype=np.float32)

    consts = _host_consts()
    in_maps = []
    for c in range(NCORES):
        b, hg = c // 2, c % 2
        fsl = slice(hg * F, (hg + 1) * F)
        in_maps.append(
            {
                "x": np.ascontiguousarray(hs[b]),
                "wq": np.ascontiguousarray(Wq[fsl]),
                "wk": np.ascontiguousarray(Wk[fsl]),
                "wv": np.ascontiguousarray(Wv[fsl]),
                "bqt": np.ascontiguousarray(bq[fsl].reshape(NPAIR, P).T),
                "bkt": np.ascontiguousarray(bk[fsl].reshape(NPAIR, P).T),
                "bvb": np.broadcast_to(bv[fsl], (P, F)).copy(),
                "maskb": np.ascontiguousarray((am[b, 0, 0] / 8.0).reshape(SCH, P).T),
                "cm": consts["cm"],
                "ident": consts["ident"],
            }
        )
    return in_maps


def assemble_out(results):
    out = np.empty((B, S, H), dtype=np.float32)
    for c in range(NCORES):
        b, hg = c // 2, c % 2
        out[b, :, hg * F : (hg + 1) * F] = results[c]["out"]
    return out


def kernel(**inputs):
    from concourse.bass_utils import run_bass_kernel_spmd

    in_maps = make_in_maps(inputs)
    nc = _get_nc()
    res = run_bass_kernel_spmd(nc, in_maps, list(range(NCORES)))
    return assemble_out(res.results)


if __name__ == "__main__":
    rng = np.random.default_rng(0)
    ins = {
        "hidden_states": rng.standard_normal((B, S, H)).astype(np.float32),
        "attention_mask": np.zeros((B, 1, 1, S), np.float32),
        "Wq": (rng.standard_normal((H, H)) / 32.0).astype(np.float32),
        "bq": np.zeros(H, np.float32),
        "Wk": (rng.standard_normal((H, H)) / 32.0).astype(np.float32),
        "bk": np.zeros(H, np.float32),
        "Wv": (rng.standard_normal((H, H)) / 32.0).astype(np.float32),
        "bv": np.zeros(H, np.float32),
    }
    o = kernel(**ins)
    print("out", o.shape, o.dtype, float(np.abs(o).max()))
